# revision 5
# baseline (speedup 1.0000x reference)
"""CodebookLoRASTELinear forward on 8 Trainium2 NeuronCores (v3).

out = x @ (W_q + D)^T with
  D   = (lora_B @ lora_A) * (alpha/rank)
  cb  = codebook / max|codebook|,  S = exp(scale_log)  (per [o, 128] group)
  q   = cb[searchsorted(midpoints(cb), (W+D)/S)],  W_q = q * S

Column-parallel sharding: W / scale / lora_B rows (out_features) split
across 8 cores; x and lora_A replicated; outputs concatenated on host.

v3: phase B runs entirely in TRANSPOSED [i%128, o] layout so the PE
transposes disappear and all quantization thresholds become immediates:
- host passes (W/S)^T; a rank-65 f32r matmul (lora + ones row against
  recipS-scaled lora_B + CC row) emits (D/S + CC)^T per group in one
  213ns instruction; a K=1 ones x S^T f32r matmul broadcasts S^T.
- ACT evacuates both PSUM tiles to SBUF; the whole elementwise chain is
  [128,512]-wide: Pool u/z/weff (tensor_tensor only -- the only legal
  Pool form), DVE a1/a12/a123 (immediate-scalar ops), ACT Sign.
- weff = (q + D/S + CC)*S^T lands directly as bf16 -- no PE transpose,
  no psum cast-copy; phase C is unchanged.
- groups are emitted pair-interleaved (stage-major across 2 groups) so
  per-engine in-order queues pipeline instead of walking one dep chain.
- 6 output-tile chains (chunk 0 + half of chunk 1) overlap phase B.
"""

import numpy as np
import sys

for _p in ("/opt/trn_rl_repo",):
    if _p not in sys.path:
        sys.path.insert(0, _p)

import ml_dtypes  # noqa: E402
import concourse.mybir as mybir  # noqa: E402
import concourse.tile as tile  # noqa: E402
from concourse import bacc  # noqa: E402
from concourse.bass_utils import run_bass_kernel_spmd  # noqa: E402
from contextlib import ExitStack  # noqa: E402

N_CORES = 8
M = 8192  # 4 * 2048 tokens
I = 4096  # in_features
O = 4096  # out_features
GROUP = 128
NG = I // GROUP  # 32 groups along i
RANK = 64
KAUG = RANK + 1  # 65: lora rank + ones row (injects the CC constant)
ALPHA_OVER_RANK = 32.0 / 64.0
OS = O // N_CORES  # 512 out features per core
NOB = OS // 128  # 4 output row blocks per core
MSB = 512  # m columns per x chunk
NMSB = M // MSB  # 16 chunks

F32 = mybir.dt.float32
F32R = mybir.dt.float32r
BF16 = mybir.dt.bfloat16
ALU = mybir.AluOpType
AF = mybir.ActivationFunctionType

_cache = {}


def _build_program(cb0, tk, dk, reps=1, variant=""):
    nc = bacc.Bacc("TRN2", target_bir_lowering=False, debug=False)

    xt_d = nc.dram_tensor("xt", [NMSB * 128, NG, MSB], BF16,
                          kind="ExternalInput").ap()
    # (W/S)^T in per-group row blocks: wnt[g*128 + i, o]
    wnt_d = nc.dram_tensor("wnt", [NG * 128, OS], F32, kind="ExternalInput").ap()
    # S^T broadcast tiles, host-replicated across partitions
    sclt_d = nc.dram_tensor("sclt", [NG * 128, OS], BF16,
                            kind="ExternalInput").ap()
    lan_d = nc.dram_tensor("lan", [KAUG, I], F32R, kind="ExternalInput").ap()
    # per-group recipS-scaled lora_B^T (+ CC row): [NG, KAUG, OS]
    lbtn_d = nc.dram_tensor("lbtn", [NG, KAUG, OS], F32R,
                            kind="ExternalInput").ap()
    out_d = nc.dram_tensor("out", [M, OS], BF16, kind="ExternalOutput").ap()

    d3_is_one = abs(float(dk[2]) - 1.0) < 1e-12
    CCq = float(cb0) + (float(dk[1]) / 2.0 if d3_is_one else 0.0)
    # u = (W+D)/S + CCq (the lora ones-row injects CCq), so compares use
    # shifted immediates t_k + CCq; z = a123 + dn recovers q + D/S exactly.
    t1, t2, t3 = (float(t) + CCq for t in tk)
    d1, d2, d3 = (float(d) for d in dk)

    with tile.TileContext(nc) as tc, ExitStack() as ctx:
        singles = ctx.enter_context(tc.tile_pool(name="singles", bufs=1))

        # lora factors first -- they gate the PE's first work
        lan_sb = singles.tile([KAUG, I], F32R)
        nc.sync.dma_start(out=lan_sb, in_=lan_d)

        # persistent effective transposed weight, one tile per group
        weff = [
            singles.tile([128, OS], BF16, tag=f"weff{g}", name=f"weff{g}")
            for g in range(NG)
        ]

        if reps > 1:
            ctx.enter_context(tc.For_i(0, reps, 1))

        xpool = ctx.enter_context(tc.tile_pool(name="xpool", bufs=2))
        wload = ctx.enter_context(tc.tile_pool(name="wload", bufs=4))
        lbpool = ctx.enter_context(tc.tile_pool(name="lbpool", bufs=4))
        dnp = ctx.enter_context(tc.tile_pool(name="dnp", bufs=4))
        sbp = ctx.enter_context(tc.tile_pool(name="sbp", bufs=4))
        upool = ctx.enter_context(tc.tile_pool(name="upool", bufs=2))
        a1p = ctx.enter_context(tc.tile_pool(name="a1p", bufs=2))
        a2p = ctx.enter_context(tc.tile_pool(name="a2p", bufs=2))
        a12p = ctx.enter_context(tc.tile_pool(name="a12p", bufs=2))
        a123p = ctx.enter_context(tc.tile_pool(name="a123p", bufs=2))
        zp = ctx.enter_context(tc.tile_pool(name="zp", bufs=2))
        opool = ctx.enter_context(tc.tile_pool(name="opool", bufs=4))
        psumD = ctx.enter_context(tc.tile_pool(name="psumD", bufs=2, space="PSUM"))
        psumO = ctx.enter_context(tc.tile_pool(name="psumO", bufs=1, space="PSUM"))

        def load_chunk(msb):
            # each chunk loads as two halves on BOTH hwdge queue engines
            # (ACT + SP) -- halves the chunk latency when the per-queue
            # issue rate, not HBM, is the limit
            t = xpool.tile([128, NG, MSB], BF16, tag="xt")
            gq = NG // 2
            for q, eng in ((0, nc.scalar), (1, nc.sync)):
                eng.dma_start(
                    out=t[:, q * gq : (q + 1) * gq, :],
                    in_=xt_d[msb * 128 : (msb + 1) * 128,
                             q * gq : (q + 1) * gq, :])
            return t

        def load_wnt(g):
            t = wload.tile([128, OS], F32, tag="wnt", name=f"wnt{g}")
            nc.sync.dma_start(out=t, in_=wnt_d[g * 128 : (g + 1) * 128, :])
            return t

        def load_lbtn(g):
            t = lbpool.tile([KAUG, OS], F32R, tag="lbtn", name=f"lbtn{g}")
            nc.sync.dma_start(out=t, in_=lbtn_d[g, :, :])
            return t

        wnt_sb = {}
        lbtn_sb = {}

        # 6 output accumulation chains interleaved with phase B
        B_CHAINS = [(0, 0), (0, 1), (0, 2), (0, 3), (1, 0), (1, 1)]
        chainsB = [
            psumO.tile([128, OS], F32, tag=f"o{i}", name=f"chain{i}")
            for i in range(len(B_CHAINS))
        ]
        CH_LAG = [0, 0, 0, 0, 10, 10]  # chunk-1 chains wait for their DMA

        dn_sb = {}  # g -> evacuated (D/S + CC)^T in SBUF
        sb_sb = {}  # g -> S^T broadcast tiles (host-replicated, DMA'd)

        def load_sclb(g):
            t = sbp.tile([128, OS], BF16, tag="sb", name=f"sbs{g}")
            nc.sync.dma_start(out=t, in_=sclt_d[g * 128 : (g + 1) * 128, :])
            sb_sb[g] = t

        def emit_mms(g):
            # (D/S + CC)^T for group g: one rank-65 f32r matmul
            d_ps = psumD.tile([128, OS], F32, tag="d", name=f"dn{g}")
            nc.tensor.matmul(
                d_ps, lhsT=lan_sb[:, g * 128 : (g + 1) * 128],
                rhs=lbtn_sb[g], start=True, stop=True,
            )
            return d_ps

        def emit_evac(g, d_ps):
            dn = dnp.tile([128, OS], F32, tag="dn", name=f"dns{g}")
            nc.scalar.copy(dn, d_ps)
            dn_sb[g] = dn

        def emit_elem(gs):
            """Stage-major elementwise for a list of groups (pipelining)."""
            u = {}
            for g in gs:  # u = (W + D)/S + CC   [Pool]
                t = upool.tile([128, OS], F32, tag="u", name=f"u{g}")
                nc.gpsimd.tensor_tensor(t, wnt_sb[g], dn_sb[g], op=ALU.add)
                u[g] = t
            a1 = {}
            for g in gs:  # (u > t1+CC)*d1   [DVE, immediates]
                t = a1p.tile([128, OS], F32, tag="a1", name=f"a1{g}")
                nc.vector.tensor_scalar(t, u[g], t1, d1,
                                        op0=ALU.is_gt, op1=ALU.mult)
                a1[g] = t
            a123 = {}
            if d3_is_one:
                s2 = {}
                for g in gs:  # Sign(u - t2)   [ACT]
                    t = a2p.tile([128, OS], F32, tag="s2", name=f"s2{g}")
                    nc.scalar.activation(t, u[g], AF.Sign, bias=-t2, scale=1.0)
                    s2[g] = t
                a12 = {}
                for g in gs:  # (d2/2)*s2 + a1   [DVE]
                    t = a12p.tile([128, OS], F32, tag="a12", name=f"a12{g}")
                    nc.vector.scalar_tensor_tensor(
                        t, s2[g], d2 / 2.0, a1[g], op0=ALU.mult, op1=ALU.add)
                    a12[g] = t
                for g in gs:  # (u > t3) + a12   [DVE]
                    t = a123p.tile([128, OS], F32, tag="a123", name=f"a123{g}")
                    nc.vector.scalar_tensor_tensor(
                        t, u[g], t3, a12[g], op0=ALU.is_gt, op1=ALU.add)
                    a123[g] = t
            else:
                a2 = {}
                for g in gs:
                    t = a2p.tile([128, OS], F32, tag="s2", name=f"a2{g}")
                    nc.vector.tensor_scalar(t, u[g], t2, d2,
                                            op0=ALU.is_gt, op1=ALU.mult)
                    a2[g] = t
                a12 = {}
                for g in gs:
                    t = a12p.tile([128, OS], F32, tag="a12", name=f"a12{g}")
                    nc.gpsimd.tensor_tensor(t, a1[g], a2[g], op=ALU.add)
                    a12[g] = t
                a3 = {}
                for g in gs:
                    t = a123p.tile([128, OS], F32, tag="a123", name=f"a3{g}")
                    nc.vector.tensor_scalar(t, u[g], t3, d3,
                                            op0=ALU.is_gt, op1=ALU.mult)
                    a3[g] = t
                for g in gs:
                    t = zp.tile([128, OS], F32, tag="z", name=f"a123b{g}")
                    nc.gpsimd.tensor_tensor(t, a12[g], a3[g], op=ALU.add)
                    a123[g] = t
            z = {}
            for g in gs:  # q + D/S + CC_resid = a123 + dn  [Pool]
                t = zp.tile([128, OS], F32, tag="z2", name=f"z{g}")
                nc.gpsimd.tensor_tensor(t, a123[g], dn_sb[g], op=ALU.add)
                z[g] = t
            for g in gs:  # W_eff^T = z * S^T -> bf16   [DVE]
                nc.vector.tensor_tensor(weff[g], z[g], sb_sb[g], op=ALU.mult)
                del dn_sb[g], sb_sb[g]

        def emit_mmB(ci, g):
            msb, mb = B_CHAINS[ci]
            xt_t = xt0 if msb == 0 else xt1
            nc.tensor.matmul(
                chainsB[ci],
                lhsT=xt_t[:, g, mb * 128 : (mb + 1) * 128],
                rhs=weff[g],
                start=(g == 0),
                stop=(g == NG - 1),
                skip_group_check=True,
            )

        # groups 0-3 land before xt0 so production never stalls; xt0
        # arrives just before the first interleaved chain matmul (M_LAG)
        for g in range(4):
            wnt_sb[g] = load_wnt(g)
            lbtn_sb[g] = load_lbtn(g)
            load_sclb(g)
        xt0 = load_chunk(0)
        xt1 = load_chunk(1)

        # pair-wise pipeline: mms+evac for pair p feed elemwise for pair
        # p-1; chains trail M_LAG groups behind production.
        M_LAG = 8
        MAXLAG = M_LAG + max(CH_LAG)
        NP = NG // 2
        for p in range(NP + 1 + (MAXLAG + 1) // 2):
            for h in range(2):
                g = 2 * p + h
                if 4 <= g + 4 < NG:
                    wnt_sb[g + 4] = load_wnt(g + 4)
                    lbtn_sb[g + 4] = load_lbtn(g + 4)
                    load_sclb(g + 4)
                if g < NG:
                    d_ps = emit_mms(g)
                    emit_evac(g, d_ps)
                if h == 1 and 1 <= p <= NP:
                    emit_elem([2 * (p - 1), 2 * (p - 1) + 1])
                for ci in range(len(B_CHAINS)):
                    gg = g - M_LAG - CH_LAG[ci]
                    if 0 <= gg < NG:
                        emit_mmB(ci, gg)

        for ci, (msb, mb) in enumerate(B_CHAINS):
            o_sb = opool.tile([128, OS], BF16, tag="o")
            nc.scalar.copy(o_sb, chainsB[ci])
            mbg = msb * NOB + mb
            nc.sync.dma_start(out=out_d[mbg * 128 : (mbg + 1) * 128, :], in_=o_sb)

        # ---- phase C: stream the remaining out-tiles ----
        rest = [(1, 2), (1, 3)] + [(msb, mb) for msb in range(2, NMSB)
                                   for mb in range(NOB)]
        xts = {0: xt0, 1: xt1, 2: load_chunk(2)}
        loaded_upto = 2

        for ti, (msb, mb) in enumerate(rest):
            if mb == 0 and msb + 1 < NMSB and msb + 1 > loaded_upto:
                xts[msb + 1] = load_chunk(msb + 1)
                loaded_upto = msb + 1
            xt_t = xts[msb]
            p_out = psumO.tile([128, OS], F32, tag=f"o{ti % len(B_CHAINS)}")
            for g in range(NG):
                nc.tensor.matmul(
                    p_out,
                    lhsT=xt_t[:, g, mb * 128 : (mb + 1) * 128],
                    rhs=weff[g],
                    start=(g == 0),
                    stop=(g == NG - 1),
                )
            o_sb = opool.tile([128, OS], BF16, tag="o")
            nc.scalar.copy(o_sb, p_out)
            mbg = msb * NOB + mb
            nc.sync.dma_start(
                out=out_d[mbg * 128 : (mbg + 1) * 128, :], in_=o_sb
            )

    nc.compile()
    return nc


def _get_program(cb0, tk, dk, reps=1, variant=""):
    key = (round(float(cb0), 9), tuple(round(float(t), 9) for t in tk),
           tuple(round(float(d), 9) for d in dk), reps, variant)
    if key not in _cache:
        _cache[key] = _build_program(cb0, tk, dk, reps, variant)
    return _cache[key]


def _codebook_consts(codebook):
    cb = np.asarray(codebook, dtype=np.float64)
    cb = cb / max(float(np.max(np.abs(cb))), 1e-8)
    tk = (cb[:-1] + cb[1:]) * 0.5
    dk = np.diff(cb)
    return float(cb[0]), [float(v) for v in tk], [float(v) for v in dk]


def _prep_in_maps(x, weight, scale_log, lora_A, lora_B, codebook=None):
    cb0, tk, dk = _codebook_consts(
        codebook if codebook is not None else np.array([-1.0, -0.25, 0.0, 1.0]))
    d3_is_one = abs(float(dk[2]) - 1.0) < 1e-12
    # CC_q shifts the a123 partial sum to the true codebook value; it is
    # injected through the lora ones-row so it never touches the compares.
    CCq = float(cb0) + (float(dk[1]) / 2.0 if d3_is_one else 0.0)

    xf = np.ascontiguousarray(x.reshape(M, I), dtype=np.float32)
    xt = (
        xf.reshape(NMSB, MSB, NG, GROUP)
        .transpose(0, 3, 2, 1)
        .astype(ml_dtypes.bfloat16)
        .reshape(NMSB * 128, NG, MSB)
    )
    scl_full = np.exp(np.ascontiguousarray(
        scale_log.reshape(O, NG), dtype=np.float64))  # [O, NG]
    lan = np.zeros((KAUG, I), dtype=np.float32)
    lan[:RANK] = np.ascontiguousarray(lora_A, dtype=np.float32)
    lan[RANK] = 1.0

    in_maps = []
    for c in range(N_CORES):
        sl = slice(c * OS, (c + 1) * OS)
        w_c = np.asarray(weight[sl], dtype=np.float64)          # [OS, I]
        scl_c = scl_full[sl]                                    # [OS, NG]
        # (W/S)^T per-group row blocks
        wns = (w_c.reshape(OS, NG, GROUP) / scl_c[:, :, None])  # [OS, NG, G]
        wnt = np.ascontiguousarray(
            wns.transpose(1, 2, 0).reshape(NG * GROUP, OS)).astype(np.float32)
        # recipS-scaled lora_B^T + CC row, per group
        lbt = (np.asarray(lora_B[sl], dtype=np.float64).T
               * ALPHA_OVER_RANK)                               # [RANK, OS]
        lbtn = np.empty((NG, KAUG, OS), dtype=np.float32)
        for g in range(NG):
            lbtn[g, :RANK] = (lbt / scl_c[:, g][None, :]).astype(np.float32)
            lbtn[g, RANK] = CCq
        in_maps.append({
            "xt": xt,
            "wnt": wnt,
            "sclt": np.ascontiguousarray(np.repeat(
                scl_c.T.astype(ml_dtypes.bfloat16)[:, None, :], 128, axis=1
            ).reshape(NG * 128, OS)),
            "lan": lan,
            "lbtn": lbtn,
        })
    return in_maps


def kernel(x, weight, scale_log, codebook, lora_A, lora_B):
    cb0, tk, dk = _codebook_consts(codebook)
    nc = _get_program(cb0, tk, dk)
    in_maps = _prep_in_maps(x, weight, scale_log, lora_A, lora_B, codebook)
    res = run_bass_kernel_spmd(nc, in_maps, core_ids=list(range(N_CORES))).results
    out = np.concatenate(
        [np.asarray(res[c]["out"]).astype(np.float32)
         for c in range(N_CORES)], axis=1)
    return out.reshape(x.shape[0], x.shape[1], O)
